# revision 34
# baseline (speedup 1.0000x reference)
"""Trainium2 Bass kernel for the DimeNet-style directed-message block.

Reference computation (W = n_angles, E = n_edges, D = 128, A = 49, J = 8):
    m_kj     = m_ji[kj_idx]                          # [W, D]
    transf_m = silu(m_kj @ W_nbr + b_nbr)            # [W, D]
    transf_e = e_rbf[kj_idx] @ W_e                   # [W, D]
    m_and_e  = transf_m * transf_e                   # [W, D]
    transf_a = a_sbf @ W_a                           # [W, J]
    out[w,i] = sum_{j,l} transf_a[w,j] m_and_e[w,l] final_w[i,j,l]
    final    = segment_sum(out, kj_idx, E)           # [E, D]

Algebraic refactor: every per-angle factor except transf_a depends on the
angle only through kj_idx, so the segment sum commutes through the bilinear
form:
    me       = silu(m_ji @ W_nbr + b) * (e_rbf @ W_e)        # [E, D]
    S        = segment_sum(a_sbf @ W_a, kj_idx, E)           # [E, J]
    final[e] = sum_j S[e,j] * (me[e] @ final_w[:,j,:].T)     # [E, D]

S is computed without any scatter for the common case: the host bins each
edge's angles into rank slots (rank r = r-th angle of its edge) and lays out
a_sbf^T so that rank pass r streams through the PE aligned by edge; PSUM
accumulation over the rank passes IS the segment sum.  A rank PAIR occupies
one 128-partition block (even rank rows 0:49, odd rank rows 64:113, zeros
elsewhere); the lhsT holds W_a twice at the same offsets so a single matmul
contracts and sums both ranks.  Edges with more than R0=8 angles spill into
compacted overflow levels whose partial sums are scatter-added
(dma_scatter_add) with *unique* indices per call into a row-permuted DRAM
accumulator (row r = (e%128)*208 + e//128) so the phase-B readback is a
contiguous 2 KB/partition stream.

All heavy streams are bf16 and shaped [128, *]: per-partition DMA
descriptors of a 128-partition transfer spread across all 16 SDMA engines,
while odd partition counts pin every descriptor to a single engine
(measured: the fp32 [113, 2048] layout serialized 42 MB on one engine at
~25 GB/s, which was the entire baseline bottleneck).

Sharding: edges are contiguous, 25000 per core; angles are binned by owner
core (kj // 25000) so scatter indices fit int16 and no collective is needed.
"""

import numpy as np
import ml_dtypes

import concourse.bass as bass
import concourse.mybir as mybir
import concourse.tile as tile
from concourse import bacc, bass_utils, library_config

F32 = mybir.dt.float32
BF16 = mybir.dt.bfloat16
I16 = mybir.dt.int16
AF = mybir.ActivationFunctionType
OP = mybir.AluOpType
BF = ml_dtypes.bfloat16

D = 128
A_DIM = 49
N_RBF = 6
N_BIL = 8
N_CORES = 8
MAX_SCATTER = 4096      # tokens per dma_scatter_add call (ring limit)


class Cfg:
    """levels: tuple of (cap_i, R_i); level 0 cap must equal e_pad."""

    def __init__(self, e_valid, e_pad, levels):
        self.e_valid = e_valid
        self.e_pad = e_pad
        self.levels = tuple(levels)
        assert e_pad % 2048 == 0
        assert levels[0][0] == e_pad
        for cap, r in levels:
            assert cap % 512 == 0 and r % 2 == 0
        self.xg = e_pad // 128           # groups of 128 edges
        self.n_chunks_b = e_pad // 1024
        # packed aT stream: one 512-col block per (level, edge-chunk, rank
        # pair); host pads the array to a 2048-col multiple.
        self.n_blocks = sum((cap // 512) * (r // 2) for cap, r in levels)
        self.at_cols = ((self.n_blocks * 512 + 2047) // 2048) * 2048

    def key(self):
        return (self.e_valid, self.e_pad, self.levels)


def build_nc(cfg: Cfg, phases=(1, 2)):
    nc = bacc.Bacc(None)
    EP = cfg.e_pad
    XG = cfg.xg

    aT = nc.dram_tensor("a_t", [128, cfg.at_cols], BF16, kind="ExternalInput")
    mjiT = nc.dram_tensor("mji_t", [D, EP], BF16, kind="ExternalInput")
    erbf = nc.dram_tensor("erbf_t", [N_RBF, EP], BF16, kind="ExternalInput")
    wnbr = nc.dram_tensor("w_nbr", [D, D], BF16, kind="ExternalInput")
    bnbr = nc.dram_tensor("b_nbr", [D, 1], F32, kind="ExternalInput")
    wes = nc.dram_tensor("w_e", [N_RBF, D], BF16, kind="ExternalInput")
    wa2 = nc.dram_tensor("w_a2", [128, N_BIL], BF16, kind="ExternalInput")
    i8d = nc.dram_tensor("i8", [N_BIL, N_BIL], BF16, kind="ExternalInput")
    t2 = nc.dram_tensor("t2", [D, N_BIL * D], BF16, kind="ExternalInput")
    idxd = {}
    for li, (cap, _r) in enumerate(cfg.levels):
        if li == 0:
            continue
        idxd[li] = nc.dram_tensor(f"idx_l{li}", [128, cap // 16], I16,
                                  kind="ExternalInput")
    # partition-major output: outd[p, g*128 + i] = final[g*128 + p, i]
    outd = nc.dram_tensor("out", [128, EP], BF16, kind="ExternalOutput")
    sovf = nc.dram_tensor("s_ovf", [EP, 64], F32)   # internal, 256B rows

    with tile.TileContext(nc) as tc:
        nc.gpsimd.load_library(library_config.mlp)
        with tc.tile_pool(name="const", bufs=1) as cp:
            wa_sb = cp.tile([128, N_BIL], BF16)
            nc.sync.dma_start(out=wa_sb[:], in_=wa2[:])
            i8_sb = cp.tile([N_BIL, N_BIL], BF16)
            nc.sync.dma_start(out=i8_sb[:], in_=i8d[:])
            wn_sb = cp.tile([D, D], BF16)
            nc.sync.dma_start(out=wn_sb[:], in_=wnbr[:])
            b_sb = cp.tile([D, 1], F32)
            nc.sync.dma_start(out=b_sb[:], in_=bnbr[:])
            we_sb = cp.tile([N_RBF, D], BF16)
            nc.sync.dma_start(out=we_sb[:], in_=wes[:])
            t2_sb = cp.tile([D, N_BIL * D], BF16)
            nc.sync.dma_start(out=t2_sb[:], in_=t2[:])
            s_sbuf = cp.tile([128, XG * N_BIL], F32)
            me_sb = cp.tile([128, EP], BF16)     # edge features, all chunks

            # ---- zero the overflow accumulator ----
            s_flat = sovf.ap().rearrange("(p x) c -> p (x c)", p=128)
            zcols = s_flat.shape[1]
            with tc.tile_pool(name="zero", bufs=1) as zp:
                zt = zp.tile([128, zcols // 4], F32)
                nc.vector.memset(zt[:], 0.0)
                for q in range(4):
                    nc.sync.dma_start(
                        out=s_flat[:, q * (zcols // 4):(q + 1) * (zcols // 4)],
                        in_=zt[:])

            # ==== fused pipeline ====
            # Overflow levels run first so the scatters land early; L0 chunks
            # then stream while the S-independent edge transform (me) and the
            # S-apply stage for already-finished chunks interleave, keeping
            # PE/ACT/DVE/DMA all busy concurrently.
            s_flat32 = sovf.ap().rearrange("(p x) c -> p (x c)", p=128)
            with tc.tile_pool(name="pa", bufs=6) as pa, \
                 tc.tile_pool(name="stp", bufs=3) as stp, \
                 tc.tile_pool(name="stage", bufs=1) as stage, \
                 tc.tile_pool(name="pbm", bufs=6) as pbm, \
                 tc.tile_pool(name="sbp", bufs=12) as sbp, \
                 tc.tile_pool(name="qsp", bufs=2) as qsp, \
                 tc.tile_pool(name="osp", bufs=6) as osp, \
                 tc.tile_pool(name="pss", bufs=1, space="PSUM") as pss, \
                 tc.tile_pool(name="pst", bufs=1, space="PSUM") as pst, \
                 tc.tile_pool(name="pwork", bufs=3, space="PSUM") as pw:
                me_done = 0

                def emit_me_chunk(c):
                    # edge transform for chunk c -> me_sb[:, c*1024:...]
                    er_t = pbm.tile([N_RBF, 1024], BF16, tag="er")
                    nc.sync.dma_start(out=er_t[:],
                                      in_=erbf[:, c * 1024:(c + 1) * 1024])
                    te_ps = pw.tile([128, 1024], F32, tag="w")
                    for n in range(2):
                        nc.tensor.matmul(
                            te_ps[:, n * 512:(n + 1) * 512],
                            we_sb[:], er_t[:, n * 512:(n + 1) * 512],
                            start=True, stop=True)
                    mj = pbm.tile([128, 1024], BF16, tag="mj")
                    nc.sync.dma_start(out=mj[:],
                                      in_=mjiT[:, c * 1024:(c + 1) * 1024])
                    tm_ps = pw.tile([128, 1024], F32, tag="w")
                    for n in range(2):
                        nc.tensor.matmul(
                            tm_ps[:, n * 512:(n + 1) * 512],
                            wn_sb[:], mj[:, n * 512:(n + 1) * 512],
                            start=True, stop=True)
                    tm_sb = pbm.tile([128, 1024], BF16, tag="tm")
                    nc.scalar.activation(tm_sb[:], tm_ps[:], AF.Silu,
                                         bias=b_sb[:, 0:1])
                    nc.vector.tensor_mul(
                        me_sb[:, c * 1024:(c + 1) * 1024], tm_sb[:], te_ps[:])

                def emit_merge(q):
                    # fold the scattered overflow S into s_sbuf, one batch of
                    # <=32 groups (matches the transpose copy batches)
                    g0, g1 = 32 * q, min(32 * (q + 1), XG)
                    qt = qsp.tile([128, 2048], F32, tag="qt", name="qt")
                    nc.scalar.dma_start(
                        out=qt[:, :(g1 - g0) * 64],
                        in_=s_flat32[:, g0 * 64:g1 * 64])
                    nc.vector.tensor_add(
                        s_sbuf[:, g0 * 8:g1 * 8]
                        .rearrange("p (g j) -> p g j", j=8),
                        s_sbuf[:, g0 * 8:g1 * 8]
                        .rearrange("p (g j) -> p g j", j=8),
                        qt[:, :(g1 - g0) * 64]
                        .rearrange("p (g j) -> p g j", j=64)[:, :, 0:8])

                apply_cnt = [0]

                def emit_apply_chunk(c):
                    # S-apply for chunk c: y = me @ t2, out = sum_j S_j * y_j
                    # (j-outer y layout: y[e, j*128+i])
                    out_sb = osp.tile([128, 1024], BF16, tag="os")
                    for tt in range(8):
                        g = c * 8 + tt
                        y = pw.tile([128, 1024], F32, tag="w")
                        lhsT = me_sb[:, g * 128:(g + 1) * 128]
                        nc.tensor.matmul(y[:, 0:512], lhsT, t2_sb[:, 0:512],
                                         start=True, stop=True)
                        nc.tensor.matmul(y[:, 512:1024], lhsT,
                                         t2_sb[:, 512:1024],
                                         start=True, stop=True)
                        # z[e, j, i] = y[e, j, i] * S[e, j]; alternate the
                        # scale between DVE (one broadcast mult) and ACT
                        # (8 per-j scaled contiguous copies) for balance
                        z = sbp.tile([128, 1024], BF16, tag="z")
                        apply_cnt[0] += 1
                        if apply_cnt[0] % 3 < 2:
                            sv = (s_sbuf[:, g * 8:(g + 1) * 8]
                                  .to_broadcast((128, N_BIL, D)))
                            nc.vector.tensor_mul(
                                z[:].rearrange("p (j i) -> p j i", j=8),
                                y[:].rearrange("p (j i) -> p j i", j=8), sv)
                        else:
                            for j in range(N_BIL):
                                nc.scalar.activation(
                                    z[:, j * D:(j + 1) * D],
                                    y[:, j * D:(j + 1) * D], AF.Copy,
                                    scale=s_sbuf[:, g * 8 + j:g * 8 + j + 1])
                        # contiguous pairwise j reduction
                        nc.vector.tensor_add(
                            z[:, 0:512], z[:, 0:512], z[:, 512:1024])
                        nc.vector.tensor_add(
                            z[:, 0:256], z[:, 0:256], z[:, 256:512])
                        nc.vector.tensor_add(
                            out_sb[:, tt * 128:(tt + 1) * 128],
                            z[:, 0:128], z[:, 128:256])
                    nc.scalar.dma_start(
                        out=outd[:, c * 1024:(c + 1) * 1024], in_=out_sb[:])

                at_tiles = {}

                def at_block(b):
                    ck = b // 4
                    if ck not in at_tiles:
                        t = pa.tile([128, 2048], BF16, tag="at")
                        nc.sync.dma_start(
                            out=t[:], in_=aT[:, ck * 2048:(ck + 1) * 2048])
                        at_tiles.clear()
                        at_tiles[ck] = t
                    off = (b % 4) * 512
                    return at_tiles[ck][:, off:off + 512]

                def emit_ps_chunk(li, cap, R, c, blk, pt_state):
                    # rank-pair matmuls + transpose for one 512-edge chunk.
                    # wa2 holds W_a at rows 0:49 and 64:113, matching the
                    # even/odd rank rows of the block, so one matmul
                    # contracts and sums both ranks of the pair.
                    n_groups = cap // 128
                    ps = pss.tile([N_BIL, 512], F32, tag="s")
                    for p in range(R // 2):
                        nc.tensor.matmul(
                            ps[:], wa_sb[:], at_block(blk),
                            start=(p == 0), stop=(p == R // 2 - 1))
                        blk += 1
                    st = stp.tile([N_BIL, 512], BF16, tag="st")
                    nc.scalar.activation(st[:], ps[:], AF.Copy)
                    for q in range(4):
                        gl = c * 4 + q
                        slot = gl % 64
                        if slot == 0:
                            pt_state[0] = pst.tile([128, 512], F32, tag="tp",
                                                   name="pt")
                        pt = pt_state[0]
                        nc.tensor.matmul(
                            pt[:, slot * 8:(slot + 1) * 8],
                            st[:, q * 128:(q + 1) * 128],
                            i8_sb[:], start=True, stop=True)
                        # copy out every 32 groups so the S-apply stage can
                        # start early in the pipeline
                        if slot % 32 == 31 or gl == n_groups - 1:
                            b0 = slot - slot % 32
                            g0 = gl - slot % 32
                            dst = s_sbuf if li == 0 else stages[li]
                            nc.scalar.activation(
                                dst[:, g0 * 8:(gl + 1) * 8],
                                pt[:, b0 * 8:(slot + 1) * 8], AF.Copy)
                    return blk

                stages = {}
                idx_sb = {}
                blk = 0
                # overflow levels first: their scatters gate every S read
                for li, (cap, R) in enumerate(cfg.levels):
                    if li == 0:
                        continue
                    n_groups = cap // 128
                    stages[li] = stage.tile([128, n_groups * N_BIL], F32,
                                            tag=f"stage{li}",
                                            name=f"stage{li}")
                    idx_sb[li] = stage.tile([128, cap // 16], I16,
                                            tag=f"idx{li}", name=f"idx{li}")
                    nc.sync.dma_start(out=idx_sb[li][:], in_=idxd[li][:])
                    blk0 = sum((cp // 512) * (r // 2)
                               for cp, r in cfg.levels[:li])
                    pt_state = [None]
                    for c in range(cap // 512):
                        blk0 = emit_ps_chunk(li, cap, R, c, blk0, pt_state)
                        if me_done < cfg.n_chunks_b:
                            emit_me_chunk(me_done)
                            me_done += 1
                    t0 = 0
                    while t0 < cap:     # unique indices per call
                        n_tok = min(MAX_SCATTER, cap - t0)
                        in_ap = stages[li][:, t0 // 128 * 8:
                                           (t0 + n_tok) // 128 * 8]
                        nc.gpsimd.dma_scatter_add(
                            out_ap=sovf[:, 0:N_BIL],
                            in_ap=in_ap.rearrange("p (c e) -> p c e",
                                                  e=N_BIL),
                            idxs_ap=idx_sb[li][:, t0 // 16:(t0 + n_tok) // 16],
                            num_idxs=n_tok,
                            num_idxs_reg=n_tok,
                            elem_size=N_BIL,
                            elem_step=64,
                            queue_num=0)
                        t0 += n_tok
                # level 0, with me + overflow-merge + S-apply interleaved
                cap0, R0 = cfg.levels[0]
                n_chunks0 = cap0 // 512
                merge_after = {8 * q + 7 for q in range((XG + 31) // 32)}
                merge_after = {min(m, n_chunks0 - 1) for m in merge_after}
                next_apply = 0
                merges = 0
                pt_state = [None]
                for k in range(n_chunks0):
                    blk = emit_ps_chunk(0, cap0, R0, k, blk, pt_state)
                    if k in merge_after:
                        emit_merge(merges)
                        merges += 1
                    for _ in range(2 if k < 16 else 1):
                        if me_done < cfg.n_chunks_b:
                            emit_me_chunk(me_done)
                            me_done += 1
                    # pace applies at one per two level-0 chunks; emitting
                    # denser overpaces S-production and serializes the PE
                    while (next_apply < cfg.n_chunks_b
                           and next_apply <= (k - 8) // 2
                           and merges > (8 * next_apply + 7) // 32):
                        emit_apply_chunk(next_apply)
                        next_apply += 1
                while me_done < cfg.n_chunks_b:
                    emit_me_chunk(me_done)
                    me_done += 1
                while next_apply < cfg.n_chunks_b:
                    emit_apply_chunk(next_apply)
                    next_apply += 1
    nc.finalize()
    return nc


# ----------------------------------------------------------------------------
# host-side sharding / unsharding
# ----------------------------------------------------------------------------

def make_cfg(kj, n_edges, ev=25_000, ep=26_624):
    n_cores = (n_edges + ev - 1) // ev
    owner = np.minimum(kj // ev, n_cores - 1)
    caps = []  # per level >=1: max count over cores
    for c in range(n_cores):
        loc = kj[owner == c] - c * ev
        cnt = np.bincount(loc, minlength=ev)
        base = 8
        li = 0
        while (cnt > base).any():
            n = int((cnt > base).sum())
            if li >= len(caps):
                caps.append(n)
            else:
                caps[li] = max(caps[li], n)
            base += 8
            li += 1
    levels = [(ep, 8)]
    for n in caps:
        levels.append((max(512, ((n + 511) // 512) * 512), 8))
    return Cfg(ev, ep, levels)


def prep_in_maps(cfg: Cfg, m_ji, nbr_list, angle_list, e_rbf, a_sbf, kj_idx,
                 W_nbr, b_nbr, W_e, W_a, final_w):
    del nbr_list, angle_list
    m_ji = np.asarray(m_ji, np.float32)
    e_rbf = np.asarray(e_rbf, np.float32)
    a_sbf = np.asarray(a_sbf, np.float32)
    kj = np.asarray(kj_idx).astype(np.int64)
    W_nbr = np.asarray(W_nbr, np.float32)
    b_nbr = np.asarray(b_nbr, np.float32)
    W_e = np.asarray(W_e, np.float32)
    W_a = np.asarray(W_a, np.float32)
    final_w = np.asarray(final_w, np.float32)

    n_edges = m_ji.shape[0]
    ev = cfg.e_valid
    ep = cfg.e_pad
    xg = cfg.xg
    n_cores = (n_edges + ev - 1) // ev
    owner = np.minimum(kj // ev, n_cores - 1)

    wa2 = np.zeros((128, N_BIL), np.float32)
    wa2[0:A_DIM] = W_a
    wa2[64:64 + A_DIM] = W_a
    # j-outer layout: t2[l, j*128+i] = final_w[i, j, l]
    t2 = np.ascontiguousarray(final_w.transpose(2, 1, 0).reshape(D, N_BIL * D))
    bn = np.ascontiguousarray(b_nbr.reshape(D, 1))
    i8 = np.eye(N_BIL, dtype=np.float32)

    in_maps = []
    for c in range(n_cores):
        sel = np.nonzero(owner == c)[0]
        loc = kj[sel] - c * ev
        order = np.argsort(loc, kind="stable")
        loc = loc[order]
        rows = sel[order]                       # a_sbf row per sorted token
        cnt = np.bincount(loc, minlength=ep)
        starts = np.concatenate([[0], np.cumsum(cnt)])

        # pack the rank-pass stream
        at = np.zeros((128, cfg.at_cols), BF)
        col = 0
        base = 0
        idx_maps = {}
        for li, (cap, R) in enumerate(cfg.levels):
            if li == 0:
                elist = np.arange(ep)
            else:
                elist = np.nonzero(cnt > base)[0]
                assert len(elist) <= cap, (li, len(elist), cap)
                el_pad = np.full(cap, ep - 1, np.int64)
                el_pad[:len(elist)] = elist
                # row-permute for contiguous phase-B readback
                el_r = (el_pad % 128) * xg + el_pad // 128
                w16 = el_r.astype(np.int16).reshape(-1, 16).T
                idx_maps[f"idx_l{li}"] = np.ascontiguousarray(
                    np.tile(w16, (8, 1)))
            # A_r [cap, 49] per rank
            a_rs = []
            for r in range(R):
                a_r = np.zeros((cap, A_DIM), np.float32)
                has = np.nonzero(cnt[elist] > base + r)[0]  # pos within elist
                tok = starts[elist[has]] + base + r
                a_r[has] = a_sbf[rows[tok]]
                a_rs.append(a_r.astype(BF))
            for cc in range(cap // 512):
                for p in range(R // 2):
                    at[0:A_DIM, col:col + 512] = \
                        a_rs[2 * p][cc * 512:(cc + 1) * 512].T
                    at[64:64 + A_DIM, col:col + 512] = \
                        a_rs[2 * p + 1][cc * 512:(cc + 1) * 512].T
                    col += 512
            base += R
        assert int(cnt.max()) <= base, "levels do not cover max multiplicity"

        e0, e1 = c * ev, min((c + 1) * ev, n_edges)
        mjiT = np.zeros((D, ep), np.float32)
        mjiT[:, :e1 - e0] = m_ji[e0:e1].T
        erbfT = np.zeros((N_RBF, ep), np.float32)
        erbfT[:, :e1 - e0] = e_rbf[e0:e1].T

        im = {
            "a_t": at, "mji_t": mjiT.astype(BF),
            "erbf_t": erbfT.astype(BF), "w_nbr": W_nbr.astype(BF),
            "b_nbr": bn, "w_e": W_e.astype(BF), "w_a2": wa2.astype(BF),
            "i8": i8.astype(BF), "t2": t2.astype(BF),
        }
        im.update(idx_maps)
        in_maps.append(im)
    return in_maps


def gather_output(cfg: Cfg, results, n_edges):
    outs = []
    ev = cfg.e_valid
    for c, r in enumerate(results):
        e0, e1 = c * ev, min((c + 1) * ev, n_edges)
        o = np.asarray(r["out"]).astype(np.float32)      # [128, EP]
        o = o.reshape(128, cfg.xg, D).transpose(1, 0, 2).reshape(cfg.e_pad, D)
        outs.append(o[:e1 - e0])
    return np.ascontiguousarray(np.concatenate(outs, axis=0))


_NC_CACHE = {}


def run_on_hw(inputs, cfg=None, trace=False, trace_cores=None):
    kj = np.asarray(inputs["kj_idx"]).astype(np.int64)
    if cfg is None:
        cfg = make_cfg(kj, inputs["m_ji"].shape[0])
    key = cfg.key()
    if key not in _NC_CACHE:
        _NC_CACHE[key] = build_nc(cfg)
    nc = _NC_CACHE[key]
    in_maps = prep_in_maps(cfg, **inputs)
    res = bass_utils.run_bass_kernel_spmd(
        nc, in_maps, core_ids=list(range(len(in_maps))),
        trace=trace, trace_cores=trace_cores)
    out = gather_output(cfg, res.results, inputs["m_ji"].shape[0])
    return out, res


def kernel(**inputs) -> np.ndarray:
    out, _ = run_on_hw(inputs)
    return out


# revision 36
# speedup vs baseline: 1.0586x; 1.0586x over previous
"""Trainium2 Bass kernel for the DimeNet-style directed-message block.

Reference computation (W = n_angles, E = n_edges, D = 128, A = 49, J = 8):
    m_kj     = m_ji[kj_idx]                          # [W, D]
    transf_m = silu(m_kj @ W_nbr + b_nbr)            # [W, D]
    transf_e = e_rbf[kj_idx] @ W_e                   # [W, D]
    m_and_e  = transf_m * transf_e                   # [W, D]
    transf_a = a_sbf @ W_a                           # [W, J]
    out[w,i] = sum_{j,l} transf_a[w,j] m_and_e[w,l] final_w[i,j,l]
    final    = segment_sum(out, kj_idx, E)           # [E, D]

Algebraic refactor: every per-angle factor except transf_a depends on the
angle only through kj_idx, so the segment sum commutes through the bilinear
form:
    me       = silu(m_ji @ W_nbr + b) * (e_rbf @ W_e)        # [E, D]
    S        = segment_sum(a_sbf @ W_a, kj_idx, E)           # [E, J]
    final[e] = sum_j S[e,j] * (me[e] @ final_w[:,j,:].T)     # [E, D]

S is computed without any scatter for the common case: the host bins each
edge's angles into rank slots (rank r = r-th angle of its edge) and lays out
a_sbf^T so that rank pass r streams through the PE aligned by edge; PSUM
accumulation over the rank passes IS the segment sum.  A rank PAIR occupies
one 128-partition block (even rank rows 0:49, odd rank rows 64:113, zeros
elsewhere); the lhsT holds W_a twice at the same offsets so a single matmul
contracts and sums both ranks.  Edges with more than R0=8 angles spill into
compacted overflow levels whose partial sums are scatter-added
(dma_scatter_add) with *unique* indices per call into a row-permuted DRAM
accumulator (row r = (e%128)*208 + e//128) so the phase-B readback is a
contiguous 2 KB/partition stream.

All heavy streams are bf16 and shaped [128, *]: per-partition DMA
descriptors of a 128-partition transfer spread across all 16 SDMA engines,
while odd partition counts pin every descriptor to a single engine
(measured: the fp32 [113, 2048] layout serialized 42 MB on one engine at
~25 GB/s, which was the entire baseline bottleneck).

Sharding: edges are contiguous, 25000 per core; angles are binned by owner
core (kj // 25000) so scatter indices fit int16 and no collective is needed.
"""

import numpy as np
import ml_dtypes

import concourse.bass as bass
import concourse.mybir as mybir
import concourse.tile as tile
from concourse import bacc, bass_utils, library_config

F32 = mybir.dt.float32
BF16 = mybir.dt.bfloat16
I16 = mybir.dt.int16
AF = mybir.ActivationFunctionType
OP = mybir.AluOpType
BF = ml_dtypes.bfloat16

D = 128
A_DIM = 49
N_RBF = 6
N_BIL = 8
N_CORES = 8
MAX_SCATTER = 4096      # tokens per dma_scatter_add call (ring limit)


class Cfg:
    """levels: tuple of (cap_i, R_i); level 0 cap must equal e_pad."""

    def __init__(self, e_valid, e_pad, levels):
        self.e_valid = e_valid
        self.e_pad = e_pad
        self.levels = tuple(levels)
        assert e_pad % 2048 == 0
        assert levels[0][0] == e_pad
        for cap, r in levels:
            assert cap % 512 == 0 and r % 2 == 0
        self.xg = e_pad // 128           # groups of 128 edges
        self.n_chunks_b = e_pad // 1024
        # packed aT stream: one 512-col block per (level, edge-chunk, rank
        # pair); host pads the array to a 2048-col multiple.
        self.n_blocks = sum((cap // 512) * (r // 2) for cap, r in levels)
        self.at_cols = ((self.n_blocks * 512 + 2047) // 2048) * 2048

    def key(self):
        return (self.e_valid, self.e_pad, self.levels)


def build_nc(cfg: Cfg, phases=(1, 2)):
    nc = bacc.Bacc(None)
    EP = cfg.e_pad
    XG = cfg.xg

    aT = nc.dram_tensor("a_t", [128, cfg.at_cols], BF16, kind="ExternalInput")
    mjiT = nc.dram_tensor("mji_t", [D, EP], BF16, kind="ExternalInput")
    erbf = nc.dram_tensor("erbf_t", [N_RBF, EP], BF16, kind="ExternalInput")
    wnbr = nc.dram_tensor("w_nbr", [D, D], BF16, kind="ExternalInput")
    bnbr = nc.dram_tensor("b_nbr", [D, 1], F32, kind="ExternalInput")
    wes = nc.dram_tensor("w_e", [N_RBF, D], BF16, kind="ExternalInput")
    wa2 = nc.dram_tensor("w_a2", [128, N_BIL], BF16, kind="ExternalInput")
    i8d = nc.dram_tensor("i8", [N_BIL, N_BIL], BF16, kind="ExternalInput")
    t2 = nc.dram_tensor("t2", [D, N_BIL * D], BF16, kind="ExternalInput")
    idxd = {}
    for li, (cap, _r) in enumerate(cfg.levels):
        if li == 0:
            continue
        idxd[li] = nc.dram_tensor(f"idx_l{li}", [128, cap // 16], I16,
                                  kind="ExternalInput")
    # partition-major output: outd[p, g*128 + i] = final[g*128 + p, i]
    outd = nc.dram_tensor("out", [128, EP], BF16, kind="ExternalOutput")
    sovf = nc.dram_tensor("s_ovf", [EP, 64], F32)   # internal, 256B rows

    with tile.TileContext(nc) as tc:
        nc.gpsimd.load_library(library_config.mlp)
        with tc.tile_pool(name="const", bufs=1) as cp:
            wa_sb = cp.tile([128, N_BIL], BF16)
            nc.sync.dma_start(out=wa_sb[:], in_=wa2[:])
            i8_sb = cp.tile([N_BIL, N_BIL], BF16)
            nc.sync.dma_start(out=i8_sb[:], in_=i8d[:])
            wn_sb = cp.tile([D, D], BF16)
            nc.sync.dma_start(out=wn_sb[:], in_=wnbr[:])
            b_sb = cp.tile([D, 1], F32)
            nc.sync.dma_start(out=b_sb[:], in_=bnbr[:])
            we_sb = cp.tile([N_RBF, D], BF16)
            nc.sync.dma_start(out=we_sb[:], in_=wes[:])
            t2_sb = cp.tile([D, N_BIL * D], BF16)
            nc.sync.dma_start(out=t2_sb[:], in_=t2[:])
            s_sbuf = cp.tile([128, XG * N_BIL], F32)
            me_sb = cp.tile([128, EP], BF16)     # edge features, all chunks

            # ---- zero the overflow accumulator ----
            s_flat = sovf.ap().rearrange("(p x) c -> p (x c)", p=128)
            zcols = s_flat.shape[1]
            with tc.tile_pool(name="zero", bufs=1) as zp:
                zt = zp.tile([128, zcols // 4], F32)
                nc.vector.memset(zt[:], 0.0)
                for q in range(4):
                    nc.sync.dma_start(
                        out=s_flat[:, q * (zcols // 4):(q + 1) * (zcols // 4)],
                        in_=zt[:])

            # ==== fused pipeline ====
            # Overflow levels run first so the scatters land early; L0 chunks
            # then stream while the S-independent edge transform (me) and the
            # S-apply stage for already-finished chunks interleave, keeping
            # PE/ACT/DVE/DMA all busy concurrently.
            s_flat32 = sovf.ap().rearrange("(p x) c -> p (x c)", p=128)
            with tc.tile_pool(name="pa", bufs=4) as pa, \
                 tc.tile_pool(name="stp", bufs=3) as stp, \
                 tc.tile_pool(name="stage", bufs=1) as stage, \
                 tc.tile_pool(name="pbm", bufs=6) as pbm, \
                 tc.tile_pool(name="sbp", bufs=12) as sbp, \
                 tc.tile_pool(name="qsp", bufs=2) as qsp, \
                 tc.tile_pool(name="osp", bufs=6) as osp, \
                 tc.tile_pool(name="pss", bufs=1, space="PSUM") as pss, \
                 tc.tile_pool(name="pst", bufs=1, space="PSUM") as pst, \
                 tc.tile_pool(name="pwork", bufs=3, space="PSUM") as pw:
                me_done = 0

                def emit_me_chunk(c):
                    # edge transform for chunk c -> me_sb[:, c*1024:...]
                    er_t = pbm.tile([N_RBF, 1024], BF16, tag="er")
                    nc.sync.dma_start(out=er_t[:],
                                      in_=erbf[:, c * 1024:(c + 1) * 1024])
                    te_ps = pw.tile([128, 1024], F32, tag="w")
                    for n in range(2):
                        nc.tensor.matmul(
                            te_ps[:, n * 512:(n + 1) * 512],
                            we_sb[:], er_t[:, n * 512:(n + 1) * 512],
                            start=True, stop=True)
                    mj = pbm.tile([128, 1024], BF16, tag="mj")
                    nc.sync.dma_start(out=mj[:],
                                      in_=mjiT[:, c * 1024:(c + 1) * 1024])
                    tm_ps = pw.tile([128, 1024], F32, tag="w")
                    for n in range(2):
                        nc.tensor.matmul(
                            tm_ps[:, n * 512:(n + 1) * 512],
                            wn_sb[:], mj[:, n * 512:(n + 1) * 512],
                            start=True, stop=True)
                    tm_sb = pbm.tile([128, 1024], BF16, tag="tm")
                    nc.scalar.activation(tm_sb[:], tm_ps[:], AF.Silu,
                                         bias=b_sb[:, 0:1])
                    nc.vector.tensor_mul(
                        me_sb[:, c * 1024:(c + 1) * 1024], tm_sb[:], te_ps[:])

                def emit_merge(q):
                    # fold the scattered overflow S into s_sbuf, one batch of
                    # <=32 groups (matches the transpose copy batches)
                    g0, g1 = 32 * q, min(32 * (q + 1), XG)
                    qt = qsp.tile([128, 2048], F32, tag="qt", name="qt")
                    nc.sync.dma_start(
                        out=qt[:, :(g1 - g0) * 64],
                        in_=s_flat32[:, g0 * 64:g1 * 64])
                    nc.vector.tensor_add(
                        s_sbuf[:, g0 * 8:g1 * 8]
                        .rearrange("p (g j) -> p g j", j=8),
                        s_sbuf[:, g0 * 8:g1 * 8]
                        .rearrange("p (g j) -> p g j", j=8),
                        qt[:, :(g1 - g0) * 64]
                        .rearrange("p (g j) -> p g j", j=64)[:, :, 0:8])

                apply_cnt = [0]

                def emit_apply_chunk(c):
                    # S-apply for chunk c: y = me @ t2, out = sum_j S_j * y_j
                    # (j-outer y layout: y[e, j*128+i])
                    out_sb = osp.tile([128, 1024], BF16, tag="os")
                    for tt in range(8):
                        g = c * 8 + tt
                        y = pw.tile([128, 1024], F32, tag="w")
                        lhsT = me_sb[:, g * 128:(g + 1) * 128]
                        nc.tensor.matmul(y[:, 0:512], lhsT, t2_sb[:, 0:512],
                                         start=True, stop=True)
                        nc.tensor.matmul(y[:, 512:1024], lhsT,
                                         t2_sb[:, 512:1024],
                                         start=True, stop=True)
                        # z[e, j, i] = y[e, j, i] * S[e, j]; alternate the
                        # scale between DVE (one broadcast mult) and ACT
                        # (8 per-j scaled contiguous copies) for balance
                        z = sbp.tile([128, 1024], BF16, tag="z")
                        apply_cnt[0] += 1
                        if apply_cnt[0] % 5 < 3:
                            sv = (s_sbuf[:, g * 8:(g + 1) * 8]
                                  .to_broadcast((128, N_BIL, D)))
                            nc.vector.tensor_mul(
                                z[:].rearrange("p (j i) -> p j i", j=8),
                                y[:].rearrange("p (j i) -> p j i", j=8), sv)
                        else:
                            for j in range(N_BIL):
                                nc.scalar.activation(
                                    z[:, j * D:(j + 1) * D],
                                    y[:, j * D:(j + 1) * D], AF.Copy,
                                    scale=s_sbuf[:, g * 8 + j:g * 8 + j + 1])
                        # contiguous pairwise j reduction
                        nc.vector.tensor_add(
                            z[:, 0:512], z[:, 0:512], z[:, 512:1024])
                        nc.vector.tensor_add(
                            z[:, 0:256], z[:, 0:256], z[:, 256:512])
                        nc.vector.tensor_add(
                            out_sb[:, tt * 128:(tt + 1) * 128],
                            z[:, 0:128], z[:, 128:256])
                    nc.sync.dma_start(
                        out=outd[:, c * 1024:(c + 1) * 1024], in_=out_sb[:])

                at_tiles = {}

                def at_block(b):
                    ck = b // 4
                    if ck not in at_tiles:
                        t = pa.tile([128, 2048], BF16, tag="at")
                        nc.sync.dma_start(
                            out=t[:], in_=aT[:, ck * 2048:(ck + 1) * 2048])
                        at_tiles.clear()
                        at_tiles[ck] = t
                    off = (b % 4) * 512
                    return at_tiles[ck][:, off:off + 512]

                def emit_ps_chunk(li, cap, R, c, blk, pt_state):
                    # rank-pair matmuls + transpose for one 512-edge chunk.
                    # wa2 holds W_a at rows 0:49 and 64:113, matching the
                    # even/odd rank rows of the block, so one matmul
                    # contracts and sums both ranks of the pair.
                    n_groups = cap // 128
                    ps = pss.tile([N_BIL, 512], F32, tag="s")
                    for p in range(R // 2):
                        nc.tensor.matmul(
                            ps[:], wa_sb[:], at_block(blk),
                            start=(p == 0), stop=(p == R // 2 - 1))
                        blk += 1
                    st = stp.tile([N_BIL, 512], BF16, tag="st")
                    nc.scalar.activation(st[:], ps[:], AF.Copy)
                    for q in range(4):
                        gl = c * 4 + q
                        slot = gl % 64
                        if slot == 0:
                            pt_state[0] = pst.tile([128, 512], F32, tag="tp",
                                                   name="pt")
                        pt = pt_state[0]
                        nc.tensor.matmul(
                            pt[:, slot * 8:(slot + 1) * 8],
                            st[:, q * 128:(q + 1) * 128],
                            i8_sb[:], start=True, stop=True)
                        # copy out every 32 groups so the S-apply stage can
                        # start early in the pipeline
                        if slot % 32 == 31 or gl == n_groups - 1:
                            b0 = slot - slot % 32
                            g0 = gl - slot % 32
                            dst = s_sbuf if li == 0 else stages[li]
                            nc.scalar.activation(
                                dst[:, g0 * 8:(gl + 1) * 8],
                                pt[:, b0 * 8:(slot + 1) * 8], AF.Copy)
                    return blk

                stages = {}
                idx_sb = {}
                blk = 0
                # overflow levels first: their scatters gate every S read
                for li, (cap, R) in enumerate(cfg.levels):
                    if li == 0:
                        continue
                    n_groups = cap // 128
                    stages[li] = stage.tile([128, n_groups * N_BIL], F32,
                                            tag=f"stage{li}",
                                            name=f"stage{li}")
                    idx_sb[li] = stage.tile([128, cap // 16], I16,
                                            tag=f"idx{li}", name=f"idx{li}")
                    nc.sync.dma_start(out=idx_sb[li][:], in_=idxd[li][:])
                    blk0 = sum((cp // 512) * (r // 2)
                               for cp, r in cfg.levels[:li])
                    pt_state = [None]
                    for c in range(cap // 512):
                        blk0 = emit_ps_chunk(li, cap, R, c, blk0, pt_state)
                        if me_done < cfg.n_chunks_b:
                            emit_me_chunk(me_done)
                            me_done += 1
                    t0 = 0
                    while t0 < cap:     # unique indices per call
                        n_tok = min(MAX_SCATTER, cap - t0)
                        in_ap = stages[li][:, t0 // 128 * 8:
                                           (t0 + n_tok) // 128 * 8]
                        nc.gpsimd.dma_scatter_add(
                            out_ap=sovf[:, 0:N_BIL],
                            in_ap=in_ap.rearrange("p (c e) -> p c e",
                                                  e=N_BIL),
                            idxs_ap=idx_sb[li][:, t0 // 16:(t0 + n_tok) // 16],
                            num_idxs=n_tok,
                            num_idxs_reg=n_tok,
                            elem_size=N_BIL,
                            elem_step=64,
                            queue_num=0)
                        t0 += n_tok
                # level 0, with me + overflow-merge + S-apply interleaved
                cap0, R0 = cfg.levels[0]
                n_chunks0 = cap0 // 512
                merge_after = {8 * q + 7 for q in range((XG + 31) // 32)}
                merge_after = {min(m, n_chunks0 - 1) for m in merge_after}
                next_apply = 0
                merges = 0
                pt_state = [None]
                for k in range(n_chunks0):
                    blk = emit_ps_chunk(0, cap0, R0, k, blk, pt_state)
                    if k in merge_after:
                        emit_merge(merges)
                        merges += 1
                    for _ in range(2 if k < 16 else 1):
                        if me_done < cfg.n_chunks_b:
                            emit_me_chunk(me_done)
                            me_done += 1
                    # pace applies at one per two level-0 chunks; emitting
                    # denser overpaces S-production and serializes the PE
                    while (next_apply < cfg.n_chunks_b
                           and next_apply <= (k - 8) // 2
                           and merges > (8 * next_apply + 7) // 32):
                        emit_apply_chunk(next_apply)
                        next_apply += 1
                while me_done < cfg.n_chunks_b:
                    emit_me_chunk(me_done)
                    me_done += 1
                while next_apply < cfg.n_chunks_b:
                    emit_apply_chunk(next_apply)
                    next_apply += 1
    nc.finalize()
    return nc


# ----------------------------------------------------------------------------
# host-side sharding / unsharding
# ----------------------------------------------------------------------------

def make_cfg(kj, n_edges, ev=25_000, ep=26_624):
    n_cores = (n_edges + ev - 1) // ev
    owner = np.minimum(kj // ev, n_cores - 1)
    caps = []  # per level >=1: max count over cores
    for c in range(n_cores):
        loc = kj[owner == c] - c * ev
        cnt = np.bincount(loc, minlength=ev)
        base = 8
        li = 0
        while (cnt > base).any():
            n = int((cnt > base).sum())
            if li >= len(caps):
                caps.append(n)
            else:
                caps[li] = max(caps[li], n)
            base += 8
            li += 1
    levels = [(ep, 8)]
    for n in caps:
        levels.append((max(512, ((n + 511) // 512) * 512), 8))
    return Cfg(ev, ep, levels)


def prep_in_maps(cfg: Cfg, m_ji, nbr_list, angle_list, e_rbf, a_sbf, kj_idx,
                 W_nbr, b_nbr, W_e, W_a, final_w):
    del nbr_list, angle_list
    m_ji = np.asarray(m_ji, np.float32)
    e_rbf = np.asarray(e_rbf, np.float32)
    a_sbf = np.asarray(a_sbf, np.float32)
    kj = np.asarray(kj_idx).astype(np.int64)
    W_nbr = np.asarray(W_nbr, np.float32)
    b_nbr = np.asarray(b_nbr, np.float32)
    W_e = np.asarray(W_e, np.float32)
    W_a = np.asarray(W_a, np.float32)
    final_w = np.asarray(final_w, np.float32)

    n_edges = m_ji.shape[0]
    ev = cfg.e_valid
    ep = cfg.e_pad
    xg = cfg.xg
    n_cores = (n_edges + ev - 1) // ev
    owner = np.minimum(kj // ev, n_cores - 1)

    wa2 = np.zeros((128, N_BIL), np.float32)
    wa2[0:A_DIM] = W_a
    wa2[64:64 + A_DIM] = W_a
    # j-outer layout: t2[l, j*128+i] = final_w[i, j, l]
    t2 = np.ascontiguousarray(final_w.transpose(2, 1, 0).reshape(D, N_BIL * D))
    bn = np.ascontiguousarray(b_nbr.reshape(D, 1))
    i8 = np.eye(N_BIL, dtype=np.float32)

    in_maps = []
    for c in range(n_cores):
        sel = np.nonzero(owner == c)[0]
        loc = kj[sel] - c * ev
        order = np.argsort(loc, kind="stable")
        loc = loc[order]
        rows = sel[order]                       # a_sbf row per sorted token
        cnt = np.bincount(loc, minlength=ep)
        starts = np.concatenate([[0], np.cumsum(cnt)])

        # pack the rank-pass stream
        at = np.zeros((128, cfg.at_cols), BF)
        col = 0
        base = 0
        idx_maps = {}
        for li, (cap, R) in enumerate(cfg.levels):
            if li == 0:
                elist = np.arange(ep)
            else:
                elist = np.nonzero(cnt > base)[0]
                assert len(elist) <= cap, (li, len(elist), cap)
                el_pad = np.full(cap, ep - 1, np.int64)
                el_pad[:len(elist)] = elist
                # row-permute for contiguous phase-B readback
                el_r = (el_pad % 128) * xg + el_pad // 128
                w16 = el_r.astype(np.int16).reshape(-1, 16).T
                idx_maps[f"idx_l{li}"] = np.ascontiguousarray(
                    np.tile(w16, (8, 1)))
            # A_r [cap, 49] per rank
            a_rs = []
            for r in range(R):
                a_r = np.zeros((cap, A_DIM), np.float32)
                has = np.nonzero(cnt[elist] > base + r)[0]  # pos within elist
                tok = starts[elist[has]] + base + r
                a_r[has] = a_sbf[rows[tok]]
                a_rs.append(a_r.astype(BF))
            for cc in range(cap // 512):
                for p in range(R // 2):
                    at[0:A_DIM, col:col + 512] = \
                        a_rs[2 * p][cc * 512:(cc + 1) * 512].T
                    at[64:64 + A_DIM, col:col + 512] = \
                        a_rs[2 * p + 1][cc * 512:(cc + 1) * 512].T
                    col += 512
            base += R
        assert int(cnt.max()) <= base, "levels do not cover max multiplicity"

        e0, e1 = c * ev, min((c + 1) * ev, n_edges)
        mjiT = np.zeros((D, ep), np.float32)
        mjiT[:, :e1 - e0] = m_ji[e0:e1].T
        erbfT = np.zeros((N_RBF, ep), np.float32)
        erbfT[:, :e1 - e0] = e_rbf[e0:e1].T

        im = {
            "a_t": at, "mji_t": mjiT.astype(BF),
            "erbf_t": erbfT.astype(BF), "w_nbr": W_nbr.astype(BF),
            "b_nbr": bn, "w_e": W_e.astype(BF), "w_a2": wa2.astype(BF),
            "i8": i8.astype(BF), "t2": t2.astype(BF),
        }
        im.update(idx_maps)
        in_maps.append(im)
    return in_maps


def gather_output(cfg: Cfg, results, n_edges):
    outs = []
    ev = cfg.e_valid
    for c, r in enumerate(results):
        e0, e1 = c * ev, min((c + 1) * ev, n_edges)
        o = np.asarray(r["out"]).astype(np.float32)      # [128, EP]
        o = o.reshape(128, cfg.xg, D).transpose(1, 0, 2).reshape(cfg.e_pad, D)
        outs.append(o[:e1 - e0])
    return np.ascontiguousarray(np.concatenate(outs, axis=0))


_NC_CACHE = {}


def run_on_hw(inputs, cfg=None, trace=False, trace_cores=None):
    kj = np.asarray(inputs["kj_idx"]).astype(np.int64)
    if cfg is None:
        cfg = make_cfg(kj, inputs["m_ji"].shape[0])
    key = cfg.key()
    if key not in _NC_CACHE:
        _NC_CACHE[key] = build_nc(cfg)
    nc = _NC_CACHE[key]
    in_maps = prep_in_maps(cfg, **inputs)
    res = bass_utils.run_bass_kernel_spmd(
        nc, in_maps, core_ids=list(range(len(in_maps))),
        trace=trace, trace_cores=trace_cores)
    out = gather_output(cfg, res.results, inputs["m_ji"].shape[0])
    return out, res


def kernel(**inputs) -> np.ndarray:
    out, _ = run_on_hw(inputs)
    return out
